# revision 22
# baseline (speedup 1.0000x reference)
"""GCN block (DGL GraphConv norm='both' + ReLU) on 8 TRN2 NeuronCores.

Strategy (SPMD, one program for all cores; per-core data via inputs):
  - Nodes/edges sharded by destination: core c owns dst rows [c*6250, (c+1)*6250).
  - The gather table is the raw bf16 x. The normalized adjacency block
    (a "scaled one-hot": value ns[src] = rsqrt(deg_out) at column
    dstl % 128) is built on the host per edge slot and DMA-streamed per
    chunk — the compute engines never have to materialize it, so the
    GpSimd gather pipeline runs at full rate.
  - Edges sorted by (src half, dst window of 128). Per (window, half) group
    the tile count is the max over the 8 cores (SPMD uniform schedule);
    within a group each 128-edge tile does one matmul
    psum[128f, 128d] += g[128e, 128f]^T @ oh[128e, 128d].
  - After each group, psum is copied (Scalar engine) / added (DVE) into the
    per-window aggW tile. As soon as a window's last group completes, the
    output chunk runs inline: PE matmul aggW^T @ W, Scalar engine
    Relu(psum * rsqrt(deg_in)) (bias fused away when b == 0, checked on
    the host), and a per-window DMA of the 128 output rows.

dma_gather indices are int16, so the table is split in two halves at row
32768; edges are grouped into two passes by source half. GpSimd descriptor
generation is strictly serial per SWDGE queue pair but pipelines up to
4-deep across the 4 queues (queue_num rotation), reaching ~2 ns/idx; the
schedule exists to keep every other engine out of that pipeline's way.
"""

import sys

if "/opt/trn_rl_repo" not in sys.path:
    sys.path.insert(0, "/opt/trn_rl_repo")

import numpy as np
import ml_dtypes

import concourse.bacc as bacc
import concourse.mybir as mybir
from concourse.bass import AP
from concourse.bass_utils import run_bass_kernel_spmd
from concourse.tile import TileContext

N = 50000          # nodes
D = 128            # feature dim
NCORES = 8
NPC = N // NCORES  # 6250 dst nodes per core

RN = 50048         # padded node count (multiple of 128)
HALF = 32768       # int16 index limit; table split [0, HALF) / [HALF, RN)

WND = 128                         # dst window width (= psum cols per group)
NW = (NPC + WND - 1) // WND       # 49 windows per core
OCH = NW                          # output chunks of 128 dst rows

GCH = 16                          # gather chunk: tiles per dma_gather call
NQ = 4                            # SWDGE queues used round-robin

F32 = mybir.dt.float32
BF16 = mybir.dt.bfloat16
I16 = mybir.dt.int16

TRACE = False            # set by test harness for profiling
LAST_RESULTS = None      # BassKernelResults of the last run


def _gather_idx_layout(vals):
    """[E] int16 -> [128, E//16] in dma_gather layout (16-wrap, 8x replicated)."""
    base = vals.reshape(-1, 16).T          # [16, E/16]
    return np.ascontiguousarray(np.tile(base, (8, 1)))


def _prep_inputs(x, edge_index, W, b):
    src = np.asarray(edge_index[0], dtype=np.int64)
    dst = np.asarray(edge_index[1], dtype=np.int64)
    E = src.shape[0]

    deg_out = np.bincount(src, minlength=N).astype(np.float64)
    deg_in = np.bincount(dst, minlength=N).astype(np.float64)
    ns = (1.0 / np.sqrt(np.maximum(deg_out, 1.0))).astype(np.float32)  # [N]
    nd = (1.0 / np.sqrt(np.maximum(deg_in, 1.0))).astype(np.float32)   # [N]

    core = dst // NPC
    dstl = dst - core * NPC
    half = (src >= HALF).astype(np.int64)
    w = dstl // WND

    # group id per (core, half, window); emit order is half-major
    gid = (core * 2 + half) * NW + w
    counts = np.bincount(gid, minlength=NCORES * 2 * NW).reshape(NCORES, 2 * NW)
    # uniform tiles per (half, window) group across cores
    T = np.maximum(0, -(-counts.max(axis=0) // 128)).astype(np.int64)  # [2*NW]
    tile_base = np.zeros(2 * NW + 1, dtype=np.int64)
    np.cumsum(T, out=tile_base[1:])
    TT = int(tile_base[-1])          # total tiles per core
    ThA = int(T[:NW].sum())          # tiles in half-0 pass
    ThB = TT - ThA

    # slot assignment: per core, edges ranked within their group
    order = np.argsort(gid, kind="stable")
    gid_s = gid[order]
    gstart = np.zeros(NCORES * 2 * NW + 1, dtype=np.int64)
    np.cumsum(counts.reshape(-1), out=gstart[1:])
    rank = np.arange(E, dtype=np.int64) - gstart[gid_s]

    core_s = core[order]
    slot = tile_base[gid_s - core_s * 2 * NW] * 128 + rank  # slot in core's schedule
    src_s = src[order]
    half_s = half[order]
    dl_s = (dstl - w * WND)[order]
    ns_s = ns[src_s]

    NSLOT = TT * 128
    idx_all = np.zeros((NCORES, NSLOT), dtype=np.int16)
    idx_all[core_s, slot] = np.where(half_s == 0, src_s, src_s - HALF).astype(np.int16)

    # host-built scaled one-hot: oh[slot, j] = ns[src] * (dstl % WND == j)
    oh_all = np.zeros((NCORES, NSLOT, WND), dtype=ml_dtypes.bfloat16)
    oh_all[core_s, slot, dl_s] = ns_s

    # tile meta shared by all cores: (window, k within group, group size)
    tile_meta = []
    for g in range(2 * NW):
        for k in range(int(T[g])):
            tile_meta.append((g % NW, k, int(T[g])))

    bias_zero = bool(np.all(np.asarray(b) == 0.0))

    # replicated tensors
    xp = np.zeros((RN, D), dtype=ml_dtypes.bfloat16)
    xp[:N] = np.asarray(x, dtype=np.float32).astype(ml_dtypes.bfloat16)
    x_dev = np.ascontiguousarray(xp)

    W_dev = np.ascontiguousarray(np.asarray(W, dtype=np.float32))
    brep = np.ascontiguousarray(
        np.tile(np.asarray(b, dtype=np.float32)[None, :], (128, 1)))

    in_maps = []
    for c in range(NCORES):
        ndp = np.zeros(OCH * 128, dtype=np.float32)
        ndp[:NPC] = nd[c * NPC:(c + 1) * NPC]
        nd_dev = np.ascontiguousarray(ndp.reshape(OCH, 128).T)  # [128, OCH]
        # oh device layout [128, TT, WND]: partition p, tile t = slot t*128+p
        oh_dev = np.ascontiguousarray(
            oh_all[c].reshape(TT, 128, WND).transpose(1, 0, 2))
        in_maps.append({
            "x_dev": x_dev,
            "ndr": nd_dev,
            "w": W_dev,
            "brep": brep,
            "oh_dev": oh_dev,
            "idx_a": _gather_idx_layout(idx_all[c, :ThA * 128]),
            "idx_b": _gather_idx_layout(idx_all[c, ThA * 128:]),
        })
    return in_maps, tile_meta, ThA, ThB, bias_zero


def _build_program(tile_meta, ThA, ThB, bias_zero):
    TT = ThA + ThB

    nc = bacc.Bacc("TRN2", target_bir_lowering=False, debug=False,
                   num_devices=NCORES, num_swdge_queues=NQ)

    x_d = nc.dram_tensor("x_dev", [RN, D], BF16, kind="ExternalInput")
    ndr_d = nc.dram_tensor("ndr", [128, OCH], F32, kind="ExternalInput")
    w_d = nc.dram_tensor("w", [D, D], F32, kind="ExternalInput")
    brep_d = nc.dram_tensor("brep", [128, D], F32, kind="ExternalInput")
    oh_d = nc.dram_tensor("oh_dev", [128, TT, WND], BF16, kind="ExternalInput")
    idx_a = nc.dram_tensor("idx_a", [128, ThA * 8], I16, kind="ExternalInput")
    idx_b = nc.dram_tensor("idx_b", [128, ThB * 8], I16, kind="ExternalInput")
    y_d = nc.dram_tensor("y", [128, OCH, D], F32, kind="ExternalOutput")

    with TileContext(nc) as tc:
        with (
            tc.tile_pool(name="const", bufs=1) as cpool,
            tc.tile_pool(name="gbuf", bufs=8) as gpool,
            tc.tile_pool(name="ohbuf", bufs=6) as opool,
            tc.tile_pool(name="agg", bufs=1) as apool,
            tc.tile_pool(name="psum", bufs=6, space="PSUM") as ppool,
            tc.tile_pool(name="psum2", bufs=2, space="PSUM") as ppool2,
        ):
            # ---- constants / small loads ----
            idx_a_sb = cpool.tile([128, ThA * 8], I16, tag="idxa")
            nc.sync.dma_start(out=idx_a_sb[:], in_=idx_a[:, :])
            idx_b_sb = cpool.tile([128, ThB * 8], I16, tag="idxb")
            nc.sync.dma_start(out=idx_b_sb[:], in_=idx_b[:, :])
            w_sb = cpool.tile([D, D], F32, tag="w")
            nc.sync.dma_start(out=w_sb[:], in_=w_d[:, :])
            ndr_sb = cpool.tile([128, OCH], F32, tag="ndr")
            nc.sync.dma_start(out=ndr_sb[:], in_=ndr_d[:, :])
            if not bias_zero:
                brep_sb = cpool.tile([128, D], F32, tag="brep")
                nc.sync.dma_start(out=brep_sb[:], in_=brep_d[:, :])

            aggW = [apool.tile([128, WND], F32, tag=f"agg{w}", name=f"aggW{w}")
                    for w in range(NW)]
            outW = [apool.tile([128, D], F32, tag=f"out{w}", name=f"outW{w}")
                    for w in range(NW)]
            touched = [False] * NW
            out_done = [False] * NW

            def emit_output(wdw):
                ps2 = ppool2.tile([128, D], F32, tag="ps2")
                nc.tensor.matmul(
                    ps2[:],
                    lhsT=aggW[wdw][:],
                    rhs=w_sb[:],
                    start=True,
                    stop=True,
                )
                if bias_zero:
                    # out = relu(ps2 * nd), on the Scalar engine
                    nc.scalar.activation(
                        outW[wdw][:], ps2[:],
                        mybir.ActivationFunctionType.Relu,
                        bias=0.0, scale=ndr_sb[:, wdw:wdw + 1],
                    )
                else:
                    nc.vector.tensor_scalar(
                        outW[wdw][:], ps2[:], ndr_sb[:, wdw:wdw + 1], None,
                        mybir.AluOpType.mult,
                    )
                    nc.vector.tensor_tensor(
                        outW[wdw][:], outW[wdw][:], brep_sb[:],
                        mybir.AluOpType.add,
                    )
                    nc.vector.tensor_scalar_max(outW[wdw][:], outW[wdw][:], 0.0)
                nc.sync.dma_start(out=y_d[:, wdw, :], in_=outW[wdw][:])
                out_done[wdw] = True

            qn = 0
            psum = None
            for is_b, idx_sb, Th, base_t, h_ap in (
                (False, idx_a_sb, ThA, 0, x_d[0:HALF, :]),
                (True, idx_b_sb, ThB, ThA, x_d[HALF:RN, :]),
            ):
                for t0 in range(0, Th, GCH):
                    nt = min(GCH, Th - t0)
                    nidx = nt * 128
                    g = gpool.tile([128, GCH, D], BF16, tag="g")
                    nc.gpsimd.dma_gather(
                        g[:, :nt, :],
                        h_ap,
                        idx_sb[:, t0 * 8:t0 * 8 + nidx // 16],
                        num_idxs=nidx,
                        num_idxs_reg=nidx,
                        elem_size=D,
                        single_packet=False,
                        queue_num=qn % NQ,
                    )
                    qn += 1
                    gt0 = base_t + t0
                    oh = opool.tile([128, GCH, WND], BF16, tag="oh")
                    nc.sync.dma_start(
                        out=oh[:, :nt, :], in_=oh_d[:, gt0:gt0 + nt, :])
                    for tl in range(nt):
                        t = gt0 + tl
                        wdw, k, Twh = tile_meta[t]
                        if k == 0:
                            psum = ppool.tile([128, WND], F32, tag="ps")
                        nc.tensor.matmul(
                            psum[:],
                            lhsT=g[:, tl, :],
                            rhs=oh[:, tl, :],
                            start=(k == 0),
                            stop=(k == Twh - 1),
                        )
                        if k == Twh - 1:
                            if not touched[wdw]:
                                nc.scalar.copy(aggW[wdw][:], psum[:])
                                touched[wdw] = True
                            else:
                                nc.vector.tensor_add(
                                    aggW[wdw][:], aggW[wdw][:], psum[:])
                            if is_b:
                                emit_output(wdw)

            for wdw in range(NW):
                if not touched[wdw]:
                    nc.vector.memset(aggW[wdw][:], 0.0)
                if not out_done[wdw]:
                    emit_output(wdw)

    nc.compile()
    return nc


def kernel(x, edge_index, W, b):
    global LAST_RESULTS
    x = np.asarray(x, dtype=np.float32)
    W = np.asarray(W, dtype=np.float32)
    b = np.asarray(b, dtype=np.float32)

    in_maps, tile_meta, ThA, ThB, bias_zero = _prep_inputs(x, edge_index, W, b)
    nc = _build_program(tile_meta, ThA, ThB, bias_zero)

    kwargs = {}
    if TRACE:
        kwargs["trace"] = True
    res = run_bass_kernel_spmd(nc, in_maps, list(range(NCORES)), **kwargs)
    LAST_RESULTS = res

    out = np.empty((N, D), dtype=np.float32)
    for c in range(NCORES):
        yc = np.asarray(res.results[c]["y"])          # [128, OCH, 128]
        rows = yc.transpose(1, 0, 2).reshape(OCH * 128, D)
        out[c * NPC:(c + 1) * NPC] = rows[:NPC]
    return out


# revision 26
# speedup vs baseline: 1.0131x; 1.0131x over previous
"""GCN block (DGL GraphConv norm='both' + ReLU) on 8 TRN2 NeuronCores.

Strategy (SPMD, one program for all cores; per-core data via inputs):
  - Nodes/edges sharded by destination: core c owns dst rows [c*6250, (c+1)*6250).
  - The gather table is the raw bf16 x. The normalized adjacency block
    (a "scaled one-hot": value ns[src] = rsqrt(deg_out) at column
    dstl % 128) is built on the host per edge slot and DMA-streamed per
    chunk — the compute engines never have to materialize it, so the
    GpSimd gather pipeline runs at full rate.
  - Edges sorted by (src half, dst window of 128). Per (window, half) group
    the tile count is the max over the 8 cores (SPMD uniform schedule);
    within a group each 128-edge tile does one matmul
    psum[128f, 128d] += g[128e, 128f]^T @ oh[128e, 128d].
  - After each group, psum is copied (Scalar engine) / added (DVE) into the
    per-window aggW tile. As soon as a window's last group completes, the
    output chunk runs inline: PE matmul aggW^T @ W, Scalar engine
    Relu(psum * rsqrt(deg_in)) (bias fused away when b == 0, checked on
    the host), and a per-window DMA of the 128 output rows.

dma_gather indices are int16, so the table is split in two halves at row
32768; edges are grouped into two passes by source half. GpSimd descriptor
generation is strictly serial per SWDGE queue pair but pipelines up to
4-deep across the 4 queues (queue_num rotation), reaching ~2 ns/idx; the
schedule exists to keep every other engine out of that pipeline's way.
"""

import sys

if "/opt/trn_rl_repo" not in sys.path:
    sys.path.insert(0, "/opt/trn_rl_repo")

import numpy as np
import ml_dtypes

import concourse.bacc as bacc
import concourse.mybir as mybir
from concourse.bass import AP
from concourse.bass_utils import run_bass_kernel_spmd
from concourse.tile import TileContext

N = 50000          # nodes
D = 128            # feature dim
NCORES = 8
NPC = N // NCORES  # 6250 dst nodes per core

RN = 50048         # padded node count (multiple of 128)
HALF = 32768       # int16 index limit; table split [0, HALF) / [HALF, RN)

WND = 128                         # dst window width (= psum cols per group)
NW = (NPC + WND - 1) // WND       # 49 windows per core
OCH = NW                          # output chunks of 128 dst rows

GCH = 16                          # gather chunk: tiles per dma_gather call
NQ = 4                            # SWDGE queues used round-robin

F32 = mybir.dt.float32
BF16 = mybir.dt.bfloat16
I16 = mybir.dt.int16

TRACE = False            # set by test harness for profiling
LAST_RESULTS = None      # BassKernelResults of the last run


def _gather_idx_layout(vals):
    """[E] int16 -> [128, E//16] in dma_gather layout (16-wrap, 8x replicated)."""
    base = vals.reshape(-1, 16).T          # [16, E/16]
    return np.ascontiguousarray(np.tile(base, (8, 1)))


def _prep_inputs(x, edge_index, W, b):
    src = np.asarray(edge_index[0], dtype=np.int64)
    dst = np.asarray(edge_index[1], dtype=np.int64)
    E = src.shape[0]

    deg_out = np.bincount(src, minlength=N).astype(np.float64)
    deg_in = np.bincount(dst, minlength=N).astype(np.float64)
    ns = (1.0 / np.sqrt(np.maximum(deg_out, 1.0))).astype(np.float32)  # [N]
    nd = (1.0 / np.sqrt(np.maximum(deg_in, 1.0))).astype(np.float32)   # [N]

    core = dst // NPC
    dstl = dst - core * NPC
    half = (src >= HALF).astype(np.int64)
    w = dstl // WND

    # group id per (core, half, window); emit order is half-major
    gid = (core * 2 + half) * NW + w
    counts = np.bincount(gid, minlength=NCORES * 2 * NW).reshape(NCORES, 2 * NW)
    # uniform tiles per (half, window) group across cores
    T = np.maximum(0, -(-counts.max(axis=0) // 128)).astype(np.int64)  # [2*NW]
    tile_base = np.zeros(2 * NW + 1, dtype=np.int64)
    np.cumsum(T, out=tile_base[1:])
    TT = int(tile_base[-1])          # total tiles per core
    ThA = int(T[:NW].sum())          # tiles in half-0 pass
    ThB = TT - ThA

    # slot assignment: per core, edges ranked within their group
    order = np.argsort(gid, kind="stable")
    gid_s = gid[order]
    gstart = np.zeros(NCORES * 2 * NW + 1, dtype=np.int64)
    np.cumsum(counts.reshape(-1), out=gstart[1:])
    rank = np.arange(E, dtype=np.int64) - gstart[gid_s]

    core_s = core[order]
    slot = tile_base[gid_s - core_s * 2 * NW] * 128 + rank  # slot in core's schedule
    src_s = src[order]
    half_s = half[order]
    dl_s = (dstl - w * WND)[order]
    ns_s = ns[src_s]

    NSLOT = TT * 128
    idx_all = np.zeros((NCORES, NSLOT), dtype=np.int16)
    idx_all[core_s, slot] = np.where(half_s == 0, src_s, src_s - HALF).astype(np.int16)

    # host-built scaled one-hot: oh[slot, j] = ns[src] * (dstl % WND == j)
    oh_all = np.zeros((NCORES, NSLOT, WND), dtype=ml_dtypes.bfloat16)
    oh_all[core_s, slot, dl_s] = ns_s

    # tile meta shared by all cores: (window, k within group, group size)
    tile_meta = []
    for g in range(2 * NW):
        for k in range(int(T[g])):
            tile_meta.append((g % NW, k, int(T[g])))

    bias_zero = bool(np.all(np.asarray(b) == 0.0))

    # replicated tensors
    xp = np.zeros((RN, D), dtype=ml_dtypes.bfloat16)
    xp[:N] = np.asarray(x, dtype=np.float32).astype(ml_dtypes.bfloat16)
    x_dev = np.ascontiguousarray(xp)

    W_dev = np.ascontiguousarray(np.asarray(W, dtype=np.float32))
    brep = np.ascontiguousarray(
        np.tile(np.asarray(b, dtype=np.float32)[None, :], (128, 1)))

    in_maps = []
    for c in range(NCORES):
        ndp = np.zeros(OCH * 128, dtype=np.float32)
        ndp[:NPC] = nd[c * NPC:(c + 1) * NPC]
        nd_dev = np.ascontiguousarray(ndp.reshape(OCH, 128).T)  # [128, OCH]
        # oh device layout [128, TT, WND]: partition p, tile t = slot t*128+p
        oh_dev = np.ascontiguousarray(
            oh_all[c].reshape(TT, 128, WND).transpose(1, 0, 2))
        in_maps.append({
            "x_dev": x_dev,
            "ndr": nd_dev,
            "w": W_dev,
            "brep": brep,
            "oh_dev": oh_dev,
            "idx_a": _gather_idx_layout(idx_all[c, :ThA * 128]),
            "idx_b": _gather_idx_layout(idx_all[c, ThA * 128:]),
        })
    return in_maps, tile_meta, ThA, ThB, bias_zero


def _build_program(tile_meta, ThA, ThB, bias_zero):
    TT = ThA + ThB

    nc = bacc.Bacc("TRN2", target_bir_lowering=False, debug=False,
                   num_devices=NCORES, num_swdge_queues=NQ)

    x_d = nc.dram_tensor("x_dev", [RN, D], BF16, kind="ExternalInput")
    ndr_d = nc.dram_tensor("ndr", [128, OCH], F32, kind="ExternalInput")
    w_d = nc.dram_tensor("w", [D, D], F32, kind="ExternalInput")
    brep_d = nc.dram_tensor("brep", [128, D], F32, kind="ExternalInput")
    oh_d = nc.dram_tensor("oh_dev", [128, TT, WND], BF16, kind="ExternalInput")
    idx_a = nc.dram_tensor("idx_a", [128, ThA * 8], I16, kind="ExternalInput")
    idx_b = nc.dram_tensor("idx_b", [128, ThB * 8], I16, kind="ExternalInput")
    y_d = nc.dram_tensor("y", [128, OCH, D], F32, kind="ExternalOutput")

    with TileContext(nc) as tc:
        with (
            tc.tile_pool(name="const", bufs=1) as cpool,
            tc.tile_pool(name="gbuf", bufs=8) as gpool,
            tc.tile_pool(name="ohbuf", bufs=6) as opool,
            tc.tile_pool(name="agg", bufs=1) as apool,
            tc.tile_pool(name="psum", bufs=5, space="PSUM") as ppool,
            tc.tile_pool(name="psum2", bufs=3, space="PSUM") as ppool2,
        ):
            # ---- constants / small loads ----
            # first gather chunk's indices in their own small tile, so the
            # pipeline starts without waiting for the full idx_a transfer
            n0 = min(GCH * 8, ThA * 8)
            idx_a0_sb = cpool.tile([128, n0], I16, tag="idxa0")
            nc.sync.dma_start(out=idx_a0_sb[:], in_=idx_a[:, 0:n0])
            idx_a_sb = cpool.tile([128, ThA * 8], I16, tag="idxa")
            nc.sync.dma_start(out=idx_a_sb[:], in_=idx_a[:, :])
            idx_b_sb = cpool.tile([128, ThB * 8], I16, tag="idxb")
            nc.sync.dma_start(out=idx_b_sb[:], in_=idx_b[:, :])
            w_sb = cpool.tile([D, D], F32, tag="w")
            nc.sync.dma_start(out=w_sb[:], in_=w_d[:, :])
            ndr_sb = cpool.tile([128, OCH], F32, tag="ndr")
            nc.sync.dma_start(out=ndr_sb[:], in_=ndr_d[:, :])
            if not bias_zero:
                brep_sb = cpool.tile([128, D], F32, tag="brep")
                nc.sync.dma_start(out=brep_sb[:], in_=brep_d[:, :])

            aggW = [apool.tile([128, WND], F32, tag=f"agg{w}", name=f"aggW{w}")
                    for w in range(NW)]
            outW = [apool.tile([128, D], F32, tag=f"out{w}", name=f"outW{w}")
                    for w in range(NW)]
            touched = [False] * NW
            out_done = [False] * NW

            def emit_output(wdw):
                ps2 = ppool2.tile([128, D], F32, tag="ps2")
                nc.tensor.matmul(
                    ps2[:],
                    lhsT=aggW[wdw][:],
                    rhs=w_sb[:],
                    start=True,
                    stop=True,
                )
                if bias_zero:
                    # out = relu(ps2 * nd), on the Scalar engine
                    nc.scalar.activation(
                        outW[wdw][:], ps2[:],
                        mybir.ActivationFunctionType.Relu,
                        bias=0.0, scale=ndr_sb[:, wdw:wdw + 1],
                    )
                else:
                    nc.vector.tensor_scalar(
                        outW[wdw][:], ps2[:], ndr_sb[:, wdw:wdw + 1], None,
                        mybir.AluOpType.mult,
                    )
                    nc.vector.tensor_tensor(
                        outW[wdw][:], outW[wdw][:], brep_sb[:],
                        mybir.AluOpType.add,
                    )
                    nc.vector.tensor_scalar_max(outW[wdw][:], outW[wdw][:], 0.0)
                nc.sync.dma_start(out=y_d[:, wdw, :], in_=outW[wdw][:])
                out_done[wdw] = True

            qn = 0
            psum = None
            for is_b, idx_sb, Th, base_t, h_ap in (
                (False, idx_a_sb, ThA, 0, x_d[0:HALF, :]),
                (True, idx_b_sb, ThB, ThA, x_d[HALF:RN, :]),
            ):
                # taper the final chunks so the post-gather pipeline drains
                # with less pile-up after the last descriptor is generated
                sizes = []
                rem = Th
                while rem > 0:
                    s = min(GCH, rem)
                    if is_b and rem <= 2 * GCH and rem > GCH // 2:
                        s = min(GCH // 2, rem)
                    sizes.append(s)
                    rem -= s
                t0 = 0
                for nt in sizes:
                    nidx = nt * 128
                    g = gpool.tile([128, GCH, D], BF16, tag="g")
                    src_idx = idx_sb[:, t0 * 8:t0 * 8 + nidx // 16]
                    if not is_b and t0 == 0:
                        src_idx = idx_a0_sb[:, 0:nidx // 16]
                    nc.gpsimd.dma_gather(
                        g[:, :nt, :],
                        h_ap,
                        src_idx,
                        num_idxs=nidx,
                        num_idxs_reg=nidx,
                        elem_size=D,
                        single_packet=False,
                        queue_num=qn % NQ,
                    )
                    qn += 1
                    gt0 = base_t + t0
                    oh = opool.tile([128, GCH, WND], BF16, tag="oh")
                    nc.sync.dma_start(
                        out=oh[:, :nt, :], in_=oh_d[:, gt0:gt0 + nt, :])
                    for tl in range(nt):
                        t = gt0 + tl
                        wdw, k, Twh = tile_meta[t]
                        if k == 0:
                            psum = ppool.tile([128, WND], F32, tag="ps")
                        nc.tensor.matmul(
                            psum[:],
                            lhsT=g[:, tl, :],
                            rhs=oh[:, tl, :],
                            start=(k == 0),
                            stop=(k == Twh - 1),
                        )
                        if k == Twh - 1:
                            if not touched[wdw]:
                                nc.scalar.copy(aggW[wdw][:], psum[:])
                                touched[wdw] = True
                            else:
                                nc.vector.tensor_add(
                                    aggW[wdw][:], aggW[wdw][:], psum[:])
                            if is_b:
                                emit_output(wdw)
                    t0 += nt

            for wdw in range(NW):
                if not touched[wdw]:
                    nc.vector.memset(aggW[wdw][:], 0.0)
                if not out_done[wdw]:
                    emit_output(wdw)

    nc.compile()
    return nc


def kernel(x, edge_index, W, b):
    global LAST_RESULTS
    x = np.asarray(x, dtype=np.float32)
    W = np.asarray(W, dtype=np.float32)
    b = np.asarray(b, dtype=np.float32)

    in_maps, tile_meta, ThA, ThB, bias_zero = _prep_inputs(x, edge_index, W, b)
    nc = _build_program(tile_meta, ThA, ThB, bias_zero)

    kwargs = {}
    if TRACE:
        kwargs["trace"] = True
    res = run_bass_kernel_spmd(nc, in_maps, list(range(NCORES)), **kwargs)
    LAST_RESULTS = res

    out = np.empty((N, D), dtype=np.float32)
    for c in range(NCORES):
        yc = np.asarray(res.results[c]["y"])          # [128, OCH, 128]
        rows = yc.transpose(1, 0, 2).reshape(OCH * 128, D)
        out[c * NPC:(c + 1) * NPC] = rows[:NPC]
    return out


# revision 28
# speedup vs baseline: 1.0229x; 1.0097x over previous
"""GCN block (DGL GraphConv norm='both' + ReLU) on 8 TRN2 NeuronCores.

Strategy (SPMD, one program for all cores; per-core data via inputs):
  - Nodes/edges sharded by destination: core c owns dst rows [c*6250, (c+1)*6250).
  - The gather table is the raw bf16 x. The normalized adjacency block
    (a "scaled one-hot": value ns[src] = rsqrt(deg_out) at column
    dstl % 128) is built on the host per edge slot and DMA-streamed per
    chunk — the compute engines never have to materialize it, so the
    GpSimd gather pipeline runs at full rate.
  - Edges sorted by (src half, dst window of 128). Per (window, half) group
    the tile count is the max over the 8 cores (SPMD uniform schedule);
    within a group each 128-edge tile does one matmul
    psum[128f, 128d] += g[128e, 128f]^T @ oh[128e, 128d].
  - After each group, psum is copied (Scalar engine) / added (DVE) into the
    per-window aggW tile. As soon as a window's last group completes, the
    output chunk runs inline: PE matmul aggW^T @ W, Scalar engine
    Relu(psum * rsqrt(deg_in)) (bias fused away when b == 0, checked on
    the host), and a per-window DMA of the 128 output rows.

dma_gather indices are int16, so the table is split in two halves at row
32768; edges are grouped into two passes by source half. GpSimd descriptor
generation is strictly serial per SWDGE queue pair but pipelines up to
4-deep across the 4 queues (queue_num rotation), reaching ~2 ns/idx; the
schedule exists to keep every other engine out of that pipeline's way.
"""

import sys

if "/opt/trn_rl_repo" not in sys.path:
    sys.path.insert(0, "/opt/trn_rl_repo")

import numpy as np
import ml_dtypes

import concourse.bacc as bacc
import concourse.mybir as mybir
from concourse.bass import AP
from concourse.bass_utils import run_bass_kernel_spmd
from concourse.tile import TileContext

N = 50000          # nodes
D = 128            # feature dim
NCORES = 8
NPC = N // NCORES  # 6250 dst nodes per core

RN = 50048         # padded node count (multiple of 128)
HALF = 32768       # int16 index limit; table split [0, HALF) / [HALF, RN)

WND = 128                         # dst window width (= psum cols per group)
NW = (NPC + WND - 1) // WND       # 49 windows per core
OCH = NW                          # output chunks of 128 dst rows

GCH = 16                          # gather chunk: tiles per dma_gather call
NQ = 4                            # SWDGE queues used round-robin

F32 = mybir.dt.float32
BF16 = mybir.dt.bfloat16
I16 = mybir.dt.int16

TRACE = False            # set by test harness for profiling
LAST_RESULTS = None      # BassKernelResults of the last run


def _gather_idx_layout(vals):
    """[E] int16 -> [128, E//16] in dma_gather layout (16-wrap, 8x replicated)."""
    base = vals.reshape(-1, 16).T          # [16, E/16]
    return np.ascontiguousarray(np.tile(base, (8, 1)))


def _prep_inputs(x, edge_index, W, b):
    src = np.asarray(edge_index[0], dtype=np.int64)
    dst = np.asarray(edge_index[1], dtype=np.int64)
    E = src.shape[0]

    deg_out = np.bincount(src, minlength=N).astype(np.float64)
    deg_in = np.bincount(dst, minlength=N).astype(np.float64)
    ns = (1.0 / np.sqrt(np.maximum(deg_out, 1.0))).astype(np.float32)  # [N]
    nd = (1.0 / np.sqrt(np.maximum(deg_in, 1.0))).astype(np.float32)   # [N]

    core = dst // NPC
    dstl = dst - core * NPC
    half = (src >= HALF).astype(np.int64)
    w = dstl // WND

    # group id per (core, half, window); emit order is half-major
    gid = (core * 2 + half) * NW + w
    counts = np.bincount(gid, minlength=NCORES * 2 * NW).reshape(NCORES, 2 * NW)
    # uniform tiles per (half, window) group across cores
    T = np.maximum(0, -(-counts.max(axis=0) // 128)).astype(np.int64)  # [2*NW]
    tile_base = np.zeros(2 * NW + 1, dtype=np.int64)
    np.cumsum(T, out=tile_base[1:])
    TT = int(tile_base[-1])          # total tiles per core
    ThA = int(T[:NW].sum())          # tiles in half-0 pass
    ThB = TT - ThA

    # slot assignment: per core, edges ranked within their group
    order = np.argsort(gid, kind="stable")
    gid_s = gid[order]
    gstart = np.zeros(NCORES * 2 * NW + 1, dtype=np.int64)
    np.cumsum(counts.reshape(-1), out=gstart[1:])
    rank = np.arange(E, dtype=np.int64) - gstart[gid_s]

    core_s = core[order]
    slot = tile_base[gid_s - core_s * 2 * NW] * 128 + rank  # slot in core's schedule
    src_s = src[order]
    half_s = half[order]
    dl_s = (dstl - w * WND)[order]
    ns_s = ns[src_s]

    NSLOT = TT * 128
    idx_all = np.zeros((NCORES, NSLOT), dtype=np.int16)
    idx_all[core_s, slot] = np.where(half_s == 0, src_s, src_s - HALF).astype(np.int16)

    # host-built scaled one-hot: oh[slot, j] = ns[src] * (dstl % WND == j)
    oh_all = np.zeros((NCORES, NSLOT, WND), dtype=ml_dtypes.bfloat16)
    oh_all[core_s, slot, dl_s] = ns_s

    # tile meta shared by all cores: (window, k within group, group size)
    tile_meta = []
    for g in range(2 * NW):
        for k in range(int(T[g])):
            tile_meta.append((g % NW, k, int(T[g])))

    bias_zero = bool(np.all(np.asarray(b) == 0.0))

    # replicated tensors
    xp = np.zeros((RN, D), dtype=ml_dtypes.bfloat16)
    xp[:N] = np.asarray(x, dtype=np.float32).astype(ml_dtypes.bfloat16)
    x_dev = np.ascontiguousarray(xp)

    W_dev = np.ascontiguousarray(np.asarray(W, dtype=np.float32))
    brep = np.ascontiguousarray(
        np.tile(np.asarray(b, dtype=np.float32)[None, :], (128, 1)))

    in_maps = []
    for c in range(NCORES):
        ndp = np.zeros(OCH * 128, dtype=np.float32)
        ndp[:NPC] = nd[c * NPC:(c + 1) * NPC]
        nd_dev = np.ascontiguousarray(ndp.reshape(OCH, 128).T)  # [128, OCH]
        # oh device layout [128, TT, WND]: partition p, tile t = slot t*128+p
        oh_dev = np.ascontiguousarray(
            oh_all[c].reshape(TT, 128, WND).transpose(1, 0, 2))
        in_maps.append({
            "x_dev": x_dev,
            "ndr": nd_dev,
            "w": W_dev,
            "brep": brep,
            "oh_dev": oh_dev,
            "idx_a": _gather_idx_layout(idx_all[c, :ThA * 128]),
            "idx_b": _gather_idx_layout(idx_all[c, ThA * 128:]),
        })
    return in_maps, tile_meta, ThA, ThB, bias_zero


def _build_program(tile_meta, ThA, ThB, bias_zero):
    TT = ThA + ThB

    nc = bacc.Bacc("TRN2", target_bir_lowering=False, debug=False,
                   num_devices=NCORES, num_swdge_queues=NQ)

    x_d = nc.dram_tensor("x_dev", [RN, D], BF16, kind="ExternalInput")
    ndr_d = nc.dram_tensor("ndr", [128, OCH], F32, kind="ExternalInput")
    w_d = nc.dram_tensor("w", [D, D], F32, kind="ExternalInput")
    brep_d = nc.dram_tensor("brep", [128, D], F32, kind="ExternalInput")
    oh_d = nc.dram_tensor("oh_dev", [128, TT, WND], BF16, kind="ExternalInput")
    idx_a = nc.dram_tensor("idx_a", [128, ThA * 8], I16, kind="ExternalInput")
    idx_b = nc.dram_tensor("idx_b", [128, ThB * 8], I16, kind="ExternalInput")
    y_d = nc.dram_tensor("y", [128, OCH, D], F32, kind="ExternalOutput")

    with TileContext(nc) as tc:
        with (
            tc.tile_pool(name="const", bufs=1) as cpool,
            tc.tile_pool(name="gbuf", bufs=8) as gpool,
            tc.tile_pool(name="ohbuf", bufs=6) as opool,
            tc.tile_pool(name="agg", bufs=1) as apool,
            tc.tile_pool(name="psum", bufs=5, space="PSUM") as ppool,
            tc.tile_pool(name="psum2", bufs=3, space="PSUM") as ppool2,
        ):
            # ---- constants / small loads ----
            # first gather chunk's indices in their own small tile, so the
            # pipeline starts without waiting for the full idx_a transfer
            n0 = min(GCH * 8, ThA * 8)
            idx_a0_sb = cpool.tile([128, n0], I16, tag="idxa0")
            nc.sync.dma_start(out=idx_a0_sb[:], in_=idx_a[:, 0:n0])
            idx_a_sb = cpool.tile([128, ThA * 8], I16, tag="idxa")
            nc.sync.dma_start(out=idx_a_sb[:], in_=idx_a[:, :])
            idx_b_sb = cpool.tile([128, ThB * 8], I16, tag="idxb")
            nc.sync.dma_start(out=idx_b_sb[:], in_=idx_b[:, :])
            w_sb = cpool.tile([D, D], F32, tag="w")
            nc.sync.dma_start(out=w_sb[:], in_=w_d[:, :])
            ndr_sb = cpool.tile([128, OCH], F32, tag="ndr")
            nc.sync.dma_start(out=ndr_sb[:], in_=ndr_d[:, :])
            if not bias_zero:
                brep_sb = cpool.tile([128, D], F32, tag="brep")
                nc.sync.dma_start(out=brep_sb[:], in_=brep_d[:, :])

            aggW = [apool.tile([128, WND], F32, tag=f"agg{w}", name=f"aggW{w}")
                    for w in range(NW)]
            outW = [apool.tile([128, D], F32, tag=f"out{w}", name=f"outW{w}")
                    for w in range(NW)]
            touched = [False] * NW
            out_done = [False] * NW

            def emit_output(wdw):
                ps2 = ppool2.tile([128, D], F32, tag="ps2")
                nc.tensor.matmul(
                    ps2[:],
                    lhsT=aggW[wdw][:],
                    rhs=w_sb[:],
                    start=True,
                    stop=True,
                )
                if bias_zero:
                    # out = relu(ps2 * nd), on the Scalar engine
                    nc.scalar.activation(
                        outW[wdw][:], ps2[:],
                        mybir.ActivationFunctionType.Relu,
                        bias=0.0, scale=ndr_sb[:, wdw:wdw + 1],
                    )
                else:
                    nc.vector.tensor_scalar(
                        outW[wdw][:], ps2[:], ndr_sb[:, wdw:wdw + 1], None,
                        mybir.AluOpType.mult,
                    )
                    nc.vector.tensor_tensor(
                        outW[wdw][:], outW[wdw][:], brep_sb[:],
                        mybir.AluOpType.add,
                    )
                    nc.vector.tensor_scalar_max(outW[wdw][:], outW[wdw][:], 0.0)
                nc.sync.dma_start(out=y_d[:, wdw, :], in_=outW[wdw][:])
                out_done[wdw] = True

            qn = 0
            psum = None
            for is_b, idx_sb, Th, base_t, h_ap in (
                (False, idx_a_sb, ThA, 0, x_d[0:HALF, :]),
                (True, idx_b_sb, ThB, ThA, x_d[HALF:RN, :]),
            ):
                # taper the final chunks so the post-gather pipeline drains
                # with less pile-up after the last descriptor is generated
                sizes = []
                rem = Th
                while rem > 0:
                    s = min(GCH, rem)
                    if is_b and rem <= 2 * GCH and rem > GCH // 2:
                        s = min(GCH // 2, rem)
                    sizes.append(s)
                    rem -= s
                t0 = 0
                for nt in sizes:
                    nidx = nt * 128
                    g = gpool.tile([128, GCH, D], BF16, tag="g")
                    src_idx = idx_sb[:, t0 * 8:t0 * 8 + nidx // 16]
                    if not is_b and t0 == 0:
                        src_idx = idx_a0_sb[:, 0:nidx // 16]
                    nc.gpsimd.dma_gather(
                        g[:, :nt, :],
                        h_ap,
                        src_idx,
                        num_idxs=nidx,
                        num_idxs_reg=nidx,
                        elem_size=D,
                        single_packet=False,
                        queue_num=qn % NQ,
                    )
                    qn += 1
                    gt0 = base_t + t0
                    oh = opool.tile([128, GCH, WND], BF16, tag="oh")
                    nc.sync.dma_start(
                        out=oh[:, :nt, :], in_=oh_d[:, gt0:gt0 + nt, :])
                    for tl in range(nt):
                        t = gt0 + tl
                        wdw, k, Twh = tile_meta[t]
                        if k == 0:
                            psum = ppool.tile([128, WND], F32, tag="ps")
                        nc.tensor.matmul(
                            psum[:],
                            lhsT=g[:, tl, :],
                            rhs=oh[:, tl, :],
                            start=(k == 0),
                            stop=(k == Twh - 1),
                        )
                        if k == Twh - 1:
                            if not touched[wdw]:
                                nc.scalar.copy(aggW[wdw][:], psum[:])
                                touched[wdw] = True
                            else:
                                nc.vector.tensor_add(
                                    aggW[wdw][:], aggW[wdw][:], psum[:])
                            if is_b:
                                emit_output(wdw)
                    t0 += nt

            for wdw in range(NW):
                if not touched[wdw]:
                    nc.vector.memset(aggW[wdw][:], 0.0)
                if not out_done[wdw]:
                    emit_output(wdw)

    nc.compile()
    return nc


def kernel(x, edge_index, W, b):
    global LAST_RESULTS
    x = np.asarray(x, dtype=np.float32)
    W = np.asarray(W, dtype=np.float32)
    b = np.asarray(b, dtype=np.float32)

    in_maps, tile_meta, ThA, ThB, bias_zero = _prep_inputs(x, edge_index, W, b)
    nc = _build_program(tile_meta, ThA, ThB, bias_zero)

    kwargs = {}
    if TRACE:
        kwargs["trace"] = True
    res = run_bass_kernel_spmd(nc, in_maps, list(range(NCORES)), **kwargs)
    LAST_RESULTS = res

    out = np.empty((N, D), dtype=np.float32)
    for c in range(NCORES):
        yc = np.asarray(res.results[c]["y"])          # [128, OCH, 128]
        rows = yc.transpose(1, 0, 2).reshape(OCH * 128, D)
        out[c * NPC:(c + 1) * NPC] = rows[:NPC]
    return out


# revision 35
# speedup vs baseline: 1.0772x; 1.0531x over previous
"""GCN block (DGL GraphConv norm='both' + ReLU) on 8 TRN2 NeuronCores.

Strategy (SPMD, one program for all cores; per-core data via inputs):
  - Nodes/edges sharded by destination: core c owns dst rows [c*6250, (c+1)*6250).
  - The gather table is the raw bf16 x. The normalized adjacency block
    (a "scaled one-hot": value ns[src] = rsqrt(deg_out) at column
    dstl % 128) is built on the host per edge slot and DMA-streamed per
    chunk — the compute engines never have to materialize it, so the
    GpSimd gather pipeline runs at full rate.
  - Edges sorted by (src half, dst window of 128). Per (window, half) group
    the tile count is the max over the 8 cores (SPMD uniform schedule);
    within a group each 128-edge tile does one matmul
    psum[128f, 128d] += g[128e, 128f]^T @ oh[128e, 128d].
  - After each group, psum is copied (Scalar engine) / added (DVE) into the
    per-window aggW tile. As soon as a window's last group completes, the
    output chunk runs inline: PE matmul aggW^T @ W, Scalar engine
    Relu(psum * rsqrt(deg_in)) (bias fused away when b == 0, checked on
    the host), and a per-window DMA of the 128 output rows.

dma_gather indices are int16, so the table is split in two halves at row
32768; edges are grouped into two passes by source half. GpSimd descriptor
generation is strictly serial per SWDGE queue pair but pipelines up to
4-deep across the 4 queues (queue_num rotation), reaching ~2 ns/idx; the
schedule exists to keep every other engine out of that pipeline's way.
"""

import sys

if "/opt/trn_rl_repo" not in sys.path:
    sys.path.insert(0, "/opt/trn_rl_repo")

import numpy as np
import ml_dtypes

import concourse.bacc as bacc
import concourse.mybir as mybir
from concourse.bass import AP
from concourse.bass_utils import run_bass_kernel_spmd
from concourse.tile import TileContext

N = 50000          # nodes
D = 128            # feature dim
NCORES = 8
NPC = N // NCORES  # 6250 dst nodes per core

RN = 50048         # padded node count (multiple of 128)
HALF = 32768       # int16 index limit; table split [0, HALF) / [HALF, RN)

WND = 128                         # dst window width (= psum cols per group)
NW = (NPC + WND - 1) // WND       # 49 windows per core
OCH = NW                          # output chunks of 128 dst rows

GCH = 8                           # gather chunk: tiles per dma_gather call
NQ = 4                            # SWDGE queues used round-robin

F32 = mybir.dt.float32
BF16 = mybir.dt.bfloat16
I16 = mybir.dt.int16

TRACE = False            # set by test harness for profiling
LAST_RESULTS = None      # BassKernelResults of the last run


def _gather_idx_layout(vals):
    """[E] int16 -> [128, E//16] in dma_gather layout (16-wrap, 8x replicated)."""
    base = vals.reshape(-1, 16).T          # [16, E/16]
    return np.ascontiguousarray(np.tile(base, (8, 1)))


def _prep_inputs(x, edge_index, W, b):
    src = np.asarray(edge_index[0], dtype=np.int64)
    dst = np.asarray(edge_index[1], dtype=np.int64)
    E = src.shape[0]

    deg_out = np.bincount(src, minlength=N).astype(np.float64)
    deg_in = np.bincount(dst, minlength=N).astype(np.float64)
    ns = (1.0 / np.sqrt(np.maximum(deg_out, 1.0))).astype(np.float32)  # [N]
    nd = (1.0 / np.sqrt(np.maximum(deg_in, 1.0))).astype(np.float32)   # [N]

    core = dst // NPC
    dstl = dst - core * NPC
    half = (src >= HALF).astype(np.int64)
    w = dstl // WND

    # group id per (core, half, window); emit order is half-major
    gid = (core * 2 + half) * NW + w
    counts = np.bincount(gid, minlength=NCORES * 2 * NW).reshape(NCORES, 2 * NW)
    # uniform tiles per (half, window) group across cores
    T = np.maximum(0, -(-counts.max(axis=0) // 128)).astype(np.int64)  # [2*NW]
    tile_base = np.zeros(2 * NW + 1, dtype=np.int64)
    np.cumsum(T, out=tile_base[1:])
    TT = int(tile_base[-1])          # total tiles per core
    ThA = int(T[:NW].sum())          # tiles in half-0 pass
    ThB = TT - ThA

    # slot assignment: per core, edges ranked within their group
    order = np.argsort(gid, kind="stable")
    gid_s = gid[order]
    gstart = np.zeros(NCORES * 2 * NW + 1, dtype=np.int64)
    np.cumsum(counts.reshape(-1), out=gstart[1:])
    rank = np.arange(E, dtype=np.int64) - gstart[gid_s]

    core_s = core[order]
    slot = tile_base[gid_s - core_s * 2 * NW] * 128 + rank  # slot in core's schedule
    src_s = src[order]
    half_s = half[order]
    dl_s = (dstl - w * WND)[order]
    ns_s = ns[src_s]

    NSLOT = TT * 128
    idx_all = np.zeros((NCORES, NSLOT), dtype=np.int16)
    idx_all[core_s, slot] = np.where(half_s == 0, src_s, src_s - HALF).astype(np.int16)

    # host-built scaled one-hot: oh[slot, j] = ns[src] * (dstl % WND == j)
    oh_all = np.zeros((NCORES, NSLOT, WND), dtype=ml_dtypes.bfloat16)
    oh_all[core_s, slot, dl_s] = ns_s

    # tile meta shared by all cores: (window, k within group, group size)
    tile_meta = []
    for g in range(2 * NW):
        for k in range(int(T[g])):
            tile_meta.append((g % NW, k, int(T[g])))

    bias_zero = bool(np.all(np.asarray(b) == 0.0))

    # replicated tensors
    xp = np.zeros((RN, D), dtype=ml_dtypes.bfloat16)
    xp[:N] = np.asarray(x, dtype=np.float32).astype(ml_dtypes.bfloat16)
    x_dev = np.ascontiguousarray(xp)

    W_dev = np.ascontiguousarray(np.asarray(W, dtype=np.float32))
    brep = np.ascontiguousarray(
        np.tile(np.asarray(b, dtype=np.float32)[None, :], (128, 1)))

    in_maps = []
    for c in range(NCORES):
        ndp = np.zeros(OCH * 128, dtype=np.float32)
        ndp[:NPC] = nd[c * NPC:(c + 1) * NPC]
        nd_dev = np.ascontiguousarray(ndp.reshape(OCH, 128).T)  # [128, OCH]
        # oh device layout [128, TT, WND]: partition p, tile t = slot t*128+p
        oh_dev = np.ascontiguousarray(
            oh_all[c].reshape(TT, 128, WND).transpose(1, 0, 2))
        in_maps.append({
            "x_dev": x_dev,
            "ndr": nd_dev,
            "w": W_dev,
            "brep": brep,
            "oh_dev": oh_dev,
            "idx_a": _gather_idx_layout(idx_all[c, :ThA * 128]),
            "idx_b": _gather_idx_layout(idx_all[c, ThA * 128:]),
        })
    return in_maps, tile_meta, ThA, ThB, bias_zero


def _build_program(tile_meta, ThA, ThB, bias_zero):
    TT = ThA + ThB

    nc = bacc.Bacc("TRN2", target_bir_lowering=False, debug=False,
                   num_devices=NCORES, num_swdge_queues=NQ)

    x_d = nc.dram_tensor("x_dev", [RN, D], BF16, kind="ExternalInput")
    ndr_d = nc.dram_tensor("ndr", [128, OCH], F32, kind="ExternalInput")
    w_d = nc.dram_tensor("w", [D, D], F32, kind="ExternalInput")
    brep_d = nc.dram_tensor("brep", [128, D], F32, kind="ExternalInput")
    oh_d = nc.dram_tensor("oh_dev", [128, TT, WND], BF16, kind="ExternalInput")
    idx_a = nc.dram_tensor("idx_a", [128, ThA * 8], I16, kind="ExternalInput")
    idx_b = nc.dram_tensor("idx_b", [128, ThB * 8], I16, kind="ExternalInput")
    y_d = nc.dram_tensor("y", [128, OCH, D], F32, kind="ExternalOutput")

    with TileContext(nc) as tc:
        with (
            tc.tile_pool(name="const", bufs=1) as cpool,
            tc.tile_pool(name="gbuf", bufs=20) as gpool,
            tc.tile_pool(name="ohbuf", bufs=6) as opool,
            tc.tile_pool(name="agg", bufs=1) as apool,
            tc.tile_pool(name="psum", bufs=5, space="PSUM") as ppool,
            tc.tile_pool(name="psum2", bufs=3, space="PSUM") as ppool2,
        ):
            # ---- constants / small loads ----
            # first gather chunk's indices in their own small tile, so the
            # pipeline starts without waiting for the full idx_a transfer
            n0 = min(GCH * 8, ThA * 8)
            idx_a0_sb = cpool.tile([128, n0], I16, tag="idxa0")
            nc.sync.dma_start(out=idx_a0_sb[:], in_=idx_a[:, 0:n0])
            idx_a_sb = cpool.tile([128, ThA * 8], I16, tag="idxa")
            nc.sync.dma_start(out=idx_a_sb[:], in_=idx_a[:, :])
            idx_b_sb = cpool.tile([128, ThB * 8], I16, tag="idxb")
            nc.sync.dma_start(out=idx_b_sb[:], in_=idx_b[:, :])
            w_sb = cpool.tile([D, D], F32, tag="w")
            nc.sync.dma_start(out=w_sb[:], in_=w_d[:, :])
            ndr_sb = cpool.tile([128, OCH], F32, tag="ndr")
            nc.sync.dma_start(out=ndr_sb[:], in_=ndr_d[:, :])
            if not bias_zero:
                brep_sb = cpool.tile([128, D], F32, tag="brep")
                nc.sync.dma_start(out=brep_sb[:], in_=brep_d[:, :])

            aggW = [apool.tile([128, WND], F32, tag=f"agg{w}", name=f"aggW{w}")
                    for w in range(NW)]
            outW = [apool.tile([128, D], F32, tag=f"out{w}", name=f"outW{w}")
                    for w in range(NW)]
            touched = [False] * NW
            out_done = [False] * NW

            def emit_output(wdw):
                ps2 = ppool2.tile([128, D], F32, tag="ps2")
                nc.tensor.matmul(
                    ps2[:],
                    lhsT=aggW[wdw][:],
                    rhs=w_sb[:],
                    start=True,
                    stop=True,
                )
                if bias_zero:
                    # out = relu(ps2 * nd), on the Scalar engine
                    nc.scalar.activation(
                        outW[wdw][:], ps2[:],
                        mybir.ActivationFunctionType.Relu,
                        bias=0.0, scale=ndr_sb[:, wdw:wdw + 1],
                    )
                else:
                    nc.vector.tensor_scalar(
                        outW[wdw][:], ps2[:], ndr_sb[:, wdw:wdw + 1], None,
                        mybir.AluOpType.mult,
                    )
                    nc.vector.tensor_tensor(
                        outW[wdw][:], outW[wdw][:], brep_sb[:],
                        mybir.AluOpType.add,
                    )
                    nc.vector.tensor_scalar_max(outW[wdw][:], outW[wdw][:], 0.0)
                nc.scalar.dma_start(out=y_d[:, wdw, :], in_=outW[wdw][:])
                out_done[wdw] = True

            qn = 0
            psum = None
            OHC = 32                   # one-hot tiles per DMA chunk
            oh_state = {"oc": -1, "buf": None}

            def oh_for(t):
                # lazily DMA the 32-tile one-hot chunk containing tile t
                oc = t // OHC
                if oc != oh_state["oc"]:
                    cnt = min(OHC, TT - oc * OHC)
                    buf = opool.tile([128, OHC, WND], BF16, tag="oh")
                    nc.sync.dma_start(
                        out=buf[:, :cnt, :],
                        in_=oh_d[:, oc * OHC:oc * OHC + cnt, :])
                    oh_state["oc"] = oc
                    oh_state["buf"] = buf
                return oh_state["buf"][:, t - oc * OHC, :]
            for is_b, idx_sb, Th, base_t, h_ap in (
                (False, idx_a_sb, ThA, 0, x_d[0:HALF, :]),
                (True, idx_b_sb, ThB, ThA, x_d[HALF:RN, :]),
            ):
                # taper the final chunks so the post-gather pipeline drains
                # with less pile-up after the last descriptor is generated
                sizes = []
                rem = Th
                while rem > 0:
                    s = min(GCH, rem)
                    if is_b and rem <= 2 * GCH and rem > GCH // 2:
                        s = min(GCH // 2, rem)
                    sizes.append(s)
                    rem -= s
                t0 = 0
                for nt in sizes:
                    nidx = nt * 128
                    g = gpool.tile([128, GCH, D], BF16, tag="g")
                    src_idx = idx_sb[:, t0 * 8:t0 * 8 + nidx // 16]
                    if not is_b and t0 == 0:
                        src_idx = idx_a0_sb[:, 0:nidx // 16]
                    nc.gpsimd.dma_gather(
                        g[:, :nt, :],
                        h_ap,
                        src_idx,
                        num_idxs=nidx,
                        num_idxs_reg=nidx,
                        elem_size=D,
                        single_packet=False,
                        queue_num=qn % NQ,
                    )
                    qn += 1
                    gt0 = base_t + t0
                    for tl in range(nt):
                        t = gt0 + tl
                        wdw, k, Twh = tile_meta[t]
                        rhs_ap = oh_for(t)
                        if k == 0:
                            psum = ppool.tile([128, WND], F32, tag="ps")
                        nc.tensor.matmul(
                            psum[:],
                            lhsT=g[:, tl, :],
                            rhs=rhs_ap,
                            start=(k == 0),
                            stop=(k == Twh - 1),
                        )
                        if k == Twh - 1:
                            if not touched[wdw]:
                                nc.scalar.copy(aggW[wdw][:], psum[:])
                                touched[wdw] = True
                            else:
                                nc.vector.tensor_add(
                                    aggW[wdw][:], aggW[wdw][:], psum[:])
                            if is_b:
                                emit_output(wdw)
                    t0 += nt

            for wdw in range(NW):
                if not touched[wdw]:
                    nc.vector.memset(aggW[wdw][:], 0.0)
                if not out_done[wdw]:
                    emit_output(wdw)

    nc.compile()
    return nc


def kernel(x, edge_index, W, b):
    global LAST_RESULTS
    x = np.asarray(x, dtype=np.float32)
    W = np.asarray(W, dtype=np.float32)
    b = np.asarray(b, dtype=np.float32)

    in_maps, tile_meta, ThA, ThB, bias_zero = _prep_inputs(x, edge_index, W, b)
    nc = _build_program(tile_meta, ThA, ThB, bias_zero)

    kwargs = {}
    if TRACE:
        kwargs["trace"] = True
    res = run_bass_kernel_spmd(nc, in_maps, list(range(NCORES)), **kwargs)
    LAST_RESULTS = res

    out = np.empty((N, D), dtype=np.float32)
    for c in range(NCORES):
        yc = np.asarray(res.results[c]["y"])          # [128, OCH, 128]
        rows = yc.transpose(1, 0, 2).reshape(OCH * 128, D)
        out[c * NPC:(c + 1) * NPC] = rows[:NPC]
    return out
